# revision 60
# baseline (speedup 1.0000x reference)
"""GCN-Attention kernel for Trainium2, data-parallel over 8 NeuronCores.

Reference computation (per image b of 64, category c of 100):
  full = concat(image_features, bbox)                    [N, 2052]
  x[b,c,:] = sum_{boxes n in bucket(b,c), slot<3} lin_w[slot]*full[n] + lin_b
  support  = x @ gc_w                                    [B, 100, 2048]
  gcn      = leaky_relu((X + adj) @ support + gc_b)
  out[b]   = global_features[b] @ gcn[b]                 [B, 2048]

Host prep (pure input reorganization, <0.3% of total FLOPs): the occurrence-
slot scatter is resolved into the weighted sum x on the host.

Algebraic restructure: the bbox columns (4) and the lin_b bias do NOT get
their own phase-2 contraction chunk.  Because
  A_b @ (x_bbox_b @ W_bbox) = (A_b @ x_bbox_b) @ W_bbox          (rank 4)
  A_b @ (lin_b * ones ⊗ colsum(W)) = lin_b * rowsum(A_b) ⊗ colsum(W)
both fold into phase 3 as 5 extra contraction rows, costing zero extra
matmuls.  Phase 2 contracts exactly K=2048 = 16 full 128-chunks.

Row packing: the 800 (image,category) rows pack into 7 partition tiles
[100,100,120x5] (matmul cost is set by the moving free dim only, so fewer
output row-tiles = fewer matmuls: 7x16x4=448 vs per-image 512; boundaries
at 100/200 are image-aligned so only 4 images straddle tiles -> 12
phase-3 pieces).  Each tile keeps 5-6 spare partitions holding the shared
extras rows [lin_b*colsum(W); W_bbox], so phase 3 streams a tile's rows
0..sz+KN as moving operand (base partition 0 — HW requires partition-0-
aligned operand access) against a per-(image,tile) stationary block
carrying the adjacency weights for that image's categories in that tile
(zeros elsewhere, extras weights on the image's first tile only).

Phases per 512-col chunk n (4 chunks):
  phase 2: tile groups {0..3} then {4..6}: per tile, 16 K=128 chunks
           x^T (stationary) x gc_w (moving), interleaved PSUM chains.
  phase 3: per image, 1-2 accumulating matmuls (K<=126) + scalar Lrelu.
  phase 4: attention row matmul [K=100 -> 1, 512], DVE copy, 2KB DMA out.
  phase-3/4 items pop between phase-2 K-chunks (from k=3), slot-gated so
  a p4 never pops before its p3's Lrelu drains (an unready matmul stalls
  the in-order PE queue); the first batch is deferred to the third unit
  so contention-delayed constant DMAs can never stall early pops; the
  final unit runs tile-major with inline item emission so the drain
  pipelines instead of trailing.  The scalar activation table is warmed
  with a dummy Lrelu at startup (a mid-run ACT_TABLE_LOAD blocks casts).

DMA: 3 rings (sync/scalar/gpsimd), consumption-ordered; first-needed
pieces split small (contiguous dram sources only — strided sources crawl)
so the first matmul issues ~4us after the engine preamble; later gc_w
quads ship whole, round-robin.  Cross-core HBM contention varies run to
run; the schedule is arranged so no popped item can block the PE queue on
a delayed transfer.
"""
import os
import time

import ml_dtypes
import numpy as np

import concourse.bacc as bacc
import concourse.mybir as mybir
import concourse.tile as tile
from concourse import bass_utils

B = 64
C = 100
LOOP = 3
FEAT = 2052
OUT = 2048
NCORES = 8
BPC = B // NCORES  # images per core
ROWS = BPC * C     # (image,category) rows per core
NKC = 16           # K chunks of 128 (image-feature contraction only)
NCH = 4            # 512-col output chunks
NT = 7             # packed row tiles; boundaries at 100/200 are image-
                   # aligned, so only 4 images straddle tiles (12 pieces)
TSZ = [100, 100, 120, 120, 120, 120, 120]
TOFF = [0, 100, 200, 320, 440, 560, 680]

f32 = mybir.dt.float32
bf16 = mybir.dt.bfloat16
np_bf16 = ml_dtypes.bfloat16

_programs: dict = {}
last_results = None  # BassKernelResults of the most recent run (for harnesses)


def _pieces():
    """Per image: list of (tile, global piece index, row range in tile).

    Piece j's stationary block lives at columns [100j, 100j+100) of adjB.
    """
    out = []
    j = 0
    for b in range(BPC):
        g0, g1 = b * C, (b + 1) * C
        ps = []
        for t in range(NT):
            lo, hi = max(g0, TOFF[t]), min(g1, TOFF[t] + TSZ[t])
            if lo < hi:
                ps.append((t, j, lo - TOFF[t], hi - TOFF[t]))
                j += 1
        out.append(ps)
    return out


PIECES = _pieces()
NPIECE = sum(len(p) for p in PIECES)  # 14


def _occ_slots(key):
    """Occurrence index among equal-valued keys, stable order (matches jax ref)."""
    n = key.shape[0]
    order = np.argsort(key, kind="stable")
    sk = key[order]
    idx = np.arange(n)
    is_new = np.concatenate([[True], sk[1:] != sk[:-1]]) if n else np.zeros(0, bool)
    run_start = np.maximum.accumulate(np.where(is_new, idx, 0))
    pos = idx - run_start
    slots = np.zeros(n, np.int64)
    slots[order] = pos
    return slots


def _build(has_gcb: bool):
    nc = bacc.Bacc("TRN2", target_bir_lowering=False, debug=False,
                   num_devices=NCORES)

    KN = 6 if has_gcb else 5   # extras rows per tile

    gcwn_d = nc.dram_tensor("gcwn", [NCH, 4, 128, OUT], bf16, kind="ExternalInput").ap()
    xtp_d = nc.dram_tensor("xtp", [NKC, 128, ROWS], bf16, kind="ExternalInput").ap()
    adjB_d = nc.dram_tensor("adjB", [127, NPIECE * C], bf16, kind="ExternalInput").ap()
    extr_d = nc.dram_tensor("extr", [KN, OUT], bf16, kind="ExternalInput").ap()
    gT_d = nc.dram_tensor("gT", [C, BPC], bf16, kind="ExternalInput").ap()
    out_d = nc.dram_tensor("out", [BPC, OUT], f32, kind="ExternalOutput").ap()

    T0 = [0, 1, 2, 3]
    T1 = [4, 5, 6]
    # images whose pieces are all within T0 tiles / needing T1 casts
    B0 = [b for b in range(BPC) if all(t <= 3 for t, _, _, _ in PIECES[b])]
    B1 = [b for b in range(BPC) if b not in B0]

    with tile.TileContext(nc) as tc:
        with tc.tile_pool(name="const", bufs=1) as cpool, \
             tc.tile_pool(name="sb", bufs=1) as pool, \
             tc.tile_pool(name="ps", bufs=1, space="PSUM") as psp:

            R = [nc.sync, nc.scalar, nc.gpsimd]

            # ---- SBUF tiles ----
            xtp_sb = [cpool.tile([128, ROWS], bf16, tag=f"xtp{k}",
                                 name=f"xtp_sb{k}") for k in range(NKC)]
            gcwn_sb = [[cpool.tile([128, OUT], bf16, tag=f"gcwn{n}{q}",
                                   name=f"gcwn_sb{n}{q}") for q in range(4)]
                       for n in range(NCH)]
            adjB_sb = cpool.tile([127, NPIECE * C], bf16, tag="adjB",
                                 name="adjB_sb")
            gT_sb = cpool.tile([C, BPC], bf16, tag="gT", name="gT_sb")
            # packed support tiles: rows 0..sz-1 = phase-2 cast; rows
            # sz..sz+KN-1 = shared extras, DMA'd once
            ssbs = [pool.tile([TSZ[t] + KN, OUT], bf16, tag="ssb", bufs=NT,
                              name=f"ssb_{t}") for t in range(NT)]
            gsbs = [pool.tile([C, OUT], bf16, tag="gsb", bufs=BPC,
                              name=f"gsb_{b}") for b in range(BPC)]

            # warm the scalar activation table with Lrelu during the
            # DMA-bound startup; otherwise the first real Lrelu (~45us)
            # triggers a 1.3us ACT_TABLE_LOAD that blocks casts and the
            # popped phase-3/4 chain behind it
            warm = pool.tile([1, 32], f32, tag="warm", name="warm")
            nc.vector.memset(warm[0:1, 0:32], 0.0)
            nc.scalar.activation(warm[0:1, 0:32], warm[0:1, 0:32],
                                 mybir.ActivationFunctionType.Lrelu,
                                 alpha=0.01)

            # warm the PE while the first input DMAs land: ~9 dummy
            # matmuls (3.8us of cold-rate activity) flip the HAM clock
            # gate to 8/8 so the first real matmuls run at 2.4GHz instead
            # of paying the 1.2GHz cold window
            wstat = pool.tile([128, 128], bf16, tag="wstat", name="wstat")
            nc.vector.memset(wstat[0:128, 0:128], 0.0)
            wmov = pool.tile([128, 512], bf16, tag="wmov", name="wmov")
            nc.vector.memset(wmov[0:128, 0:512], 0.0)
            wps = psp.tile([128, 512], f32, tag="gp", bufs=2, name="wps")
            for i in range(9):
                nc.tensor.matmul(wps[0:128, 0:512], wstat[0:128, 0:128],
                                 wmov[0:128, 0:512],
                                 start=(i == 0), stop=(i == 8))

            # ---- DMA delivery, consumption-ordered ----
            nc.sync.dma_start(xtp_sb[0][0:64, :], xtp_d[0, 0:64])
            nc.scalar.dma_start(xtp_sb[0][64:128, :], xtp_d[0, 64:128])
            nc.gpsimd.dma_start(gcwn_sb[0][0][:, 0:512], gcwn_d[0, 0, :, 0:512])
            nc.sync.dma_start(gcwn_sb[0][0][:, 512:1024],
                              gcwn_d[0, 0, :, 512:1024])
            nc.scalar.dma_start(gcwn_sb[0][0][:, 1024:1536],
                                gcwn_d[0, 0, :, 1024:1536])
            nc.gpsimd.dma_start(gcwn_sb[0][0][:, 1536:2048],
                                gcwn_d[0, 0, :, 1536:2048])
            # the tiny phase-3 constants go right behind the first-matmul
            # pieces: as early ring entries with fresh semaphores they are
            # guaranteed resident by ~14us, so a contention-delayed ring
            # can never stall the first popped phase-3 items (observed
            # costing 7-9us when these trailed the bulk stream)
            for t in range(NT):
                nc.scalar.dma_start(ssbs[t][TSZ[t]:TSZ[t] + KN, :],
                                    extr_d[:])
            nc.gpsimd.dma_start(gT_sb[:], gT_d[:])
            nc.sync.dma_start(xtp_sb[1][:], xtp_d[1])
            nc.gpsimd.dma_start(xtp_sb[2][:], xtp_d[2])
            nc.gpsimd.dma_start(xtp_sb[3][:], xtp_d[3])
            nc.sync.dma_start(adjB_sb[:], adjB_d[:])
            nc.sync.dma_start(gcwn_sb[0][1][:, 0:1024], gcwn_d[0, 1, :, 0:1024])
            nc.scalar.dma_start(gcwn_sb[0][1][:, 1024:2048],
                                gcwn_d[0, 1, :, 1024:2048])
            for k in range(4, 7):
                R[k % 3].dma_start(xtp_sb[k][:], xtp_d[k])
            nc.gpsimd.dma_start(gcwn_sb[0][2][:, 0:1024], gcwn_d[0, 2, :, 0:1024])
            nc.sync.dma_start(gcwn_sb[0][2][:, 1024:2048],
                              gcwn_d[0, 2, :, 1024:2048])
            for k in range(7, 10):
                R[k % 3].dma_start(xtp_sb[k][:], xtp_d[k])
            nc.scalar.dma_start(gcwn_sb[0][3][:, 0:1024], gcwn_d[0, 3, :, 0:1024])
            nc.gpsimd.dma_start(gcwn_sb[0][3][:, 1024:2048],
                                gcwn_d[0, 3, :, 1024:2048])
            for k in range(10, NKC):
                R[k % 3].dma_start(xtp_sb[k][:], xtp_d[k])
            # remaining gc_w quads: n=1 in halves, n=2/3 whole, round-robin
            qi = 0
            for q in range(4):
                R[qi % 3].dma_start(gcwn_sb[1][q][:, 0:1024],
                                    gcwn_d[1, q, :, 0:1024])
                R[(qi + 1) % 3].dma_start(gcwn_sb[1][q][:, 1024:2048],
                                          gcwn_d[1, q, :, 1024:2048])
                qi += 2
            for n in range(2, NCH):
                for q in range(4):
                    R[qi % 3].dma_start(gcwn_sb[n][q][:], gcwn_d[n, q])
                    qi += 1

            def stat_slice(k, t):
                return xtp_sb[k][0:128, TOFF[t]:TOFF[t] + TSZ[t]]

            def mov_slice(k, n):
                gt = gcwn_sb[n][k // 4]
                return gt[0:128, (k % 4) * 512:(k % 4) * 512 + 512]

            def cast(i, dst, src):
                # PSUM -> SBUF bf16 drain, all on the DVE: the scalar
                # engine must stay free for the Lrelu bursts when phase-3/4
                # items drain (casts queued ahead of Lrelus were measured
                # adding ~2us of PE wait per popped item group).  Chain-bank
                # reuse skips a whole unit (ch bufs=6, 4+3 chains), so the
                # serialized casts never gate the next unit's start.
                nc.vector.tensor_copy(dst, src)

            def walk(g, n, pop_item):
                # tiles of g x 16 K chunks at a fixed 512-col block:
                # interleaved PSUM chains; pending phase-3/4 items are
                # sandwiched between K chunks (from k=3, so the previous
                # unit's casts have drained) so the PE never idles
                chains = [psp.tile([128, 512], f32, tag="ch", bufs=6,
                                   name=f"ch_{t}_{n}") for t in g]
                for k in range(NKC):
                    for i, t in enumerate(g):
                        nc.tensor.matmul(
                            chains[i][0:TSZ[t], 0:512],
                            stat_slice(k, t),
                            mov_slice(k, n),
                            start=(k == 0), stop=(k == NKC - 1),
                        )
                    if k >= 3:
                        pop_item()
                for i, t in enumerate(g):
                    cast(i, ssbs[t][0:TSZ[t], n * 512:(n + 1) * 512],
                         chains[i][0:TSZ[t], 0:512])

            def walk_tmajor(g, n, pop_item, enq):
                # steady-state units (all data resident): tile-major — 16
                # back-to-back accumulating matmuls on one bank, casting
                # each tile as it completes and enqueueing the phase-3/4 of
                # images it finishes; casts spread one-per-tile instead of
                # bunching at unit boundaries (where chain-start matmuls
                # would wait on them for PSUM bank reuse)
                chains = [psp.tile([128, 512], f32, tag="ch", bufs=6,
                                   name=f"chl_{t}_{n}") for t in g]
                for i, t in enumerate(g):
                    for k in range(NKC):
                        nc.tensor.matmul(
                            chains[i][0:TSZ[t], 0:512],
                            stat_slice(k, t),
                            mov_slice(k, n),
                            start=(k == 0), stop=(k == NKC - 1),
                        )
                        if k % 3 == 2:
                            pop_item()
                    cast(i, ssbs[t][0:TSZ[t], n * 512:(n + 1) * 512],
                         chains[i][0:TSZ[t], 0:512])
                    enq(t, n)

            def p3_item(b, n):
                # G[b][:,n] = Lrelu(sum over pieces of block contraction)
                gp = psp.tile([128, 512], f32, tag="gp", bufs=2,
                              name=f"gp_{b}_{n}")
                ps = PIECES[b]
                for idx, (t, j, _, _) in enumerate(ps):
                    kk = TSZ[t] + KN
                    nc.tensor.matmul(
                        gp[0:C, 0:512],
                        adjB_sb[0:kk, j * C:(j + 1) * C],
                        ssbs[t][0:kk, n * 512:(n + 1) * 512],
                        start=(idx == 0), stop=(idx == len(ps) - 1),
                    )
                nc.scalar.activation(
                    gsbs[b][0:C, n * 512:(n + 1) * 512],
                    gp[0:C, 0:512],
                    mybir.ActivationFunctionType.Lrelu, alpha=0.01,
                )

            def p4_item(b, n):
                op = psp.tile([128, 512], f32, tag="gp", bufs=2,
                              name=f"op_{b}_{n}")
                nc.tensor.matmul(op[0:1, 0:512],
                                 gT_sb[0:C, b:b + 1],
                                 gsbs[b][0:C, n * 512:(n + 1) * 512],
                                 start=True, stop=True)
                ost = pool.tile([1, 512], f32, tag="ostage", bufs=4,
                                name=f"ost_{b}_{n}")
                nc.vector.tensor_copy(ost[0:1, 0:512], op[0:1, 0:512])
                R[b % 3].dma_start(out_d[b:b + 1, n * 512:(n + 1) * 512],
                                   ost[0:1, 0:512])

            queue = []
            slot = [0]

            def pop_item():
                # items gate on a minimum slot so a p4 never pops before its
                # p3's Lrelu (~1.3us) has drained — an unready p4 matmul
                # stalls the whole in-order PE queue
                slot[0] += 1
                if queue and queue[0][0] <= slot[0]:
                    _, kind, b, n = queue.pop(0)
                    if kind == 3:
                        p3_item(b, n)
                        queue.append((slot[0] + 2, 4, b, n))
                    else:
                        p4_item(b, n)

            # image is ready once its last tile is cast
            last_tile = {b: max(t for t, _, _, _ in PIECES[b])
                         for b in range(BPC)}

            def enq(t, n):
                for b in range(BPC):
                    if last_tile[b] == t:
                        queue.append((slot[0] + 1, 3, b, n))

            units = [(g, n) for n in range(NCH) for g in (T0, T1)]
            for u, (g, n) in enumerate(units):
                # the first phase-3 batch waits until u=2 (~40us) so a
                # contention-delayed adjB/extras DMA can never stall the
                # in-order PE queue through an early popped item
                if u == 2:
                    queue.extend((0, 3, b, 0) for b in B0)
                    queue.extend((0, 3, b, 0) for b in B1)
                elif u >= 3:
                    gp_, np_ = units[u - 1]
                    if gp_ is T0:
                        queue.extend((0, 3, b, np_) for b in B0)
                    else:
                        queue.extend((0, 3, b, np_) for b in B1)
                if u == len(units) - 1:
                    walk_tmajor(g, n, pop_item, enq)
                else:
                    walk(g, n, pop_item)
            while queue:
                pop_item()

    nc.compile()
    return nc


def _get_program(has_gcb: bool = False):
    key = ("pack115", has_gcb)
    if key not in _programs:
        _programs[key] = _build(has_gcb)
    return _programs[key]


def kernel(**inputs) -> np.ndarray:
    global last_results

    imf = np.asarray(inputs["image_features"], np.float32)
    bbox = np.asarray(inputs["bbox_list"], np.float32)
    gf = np.asarray(inputs["global_features"], np.float32)
    adj = np.asarray(inputs["adj"], np.float32)
    X = np.asarray(inputs["X"], np.float32)
    lin_w = np.asarray(inputs["lin_w"], np.float32)
    lin_b = np.float32(np.asarray(inputs["lin_b"]))
    gc_w = np.ascontiguousarray(np.asarray(inputs["gc_w"], np.float32))
    gc_b = np.asarray(inputs["gc_b"], np.float32)
    label = np.asarray(inputs["label_list"]).astype(np.int64)
    batch = np.asarray(inputs["batch"]).astype(np.int64)

    full = np.concatenate([imf, bbox], axis=1)

    # scatter bookkeeping, matching jax semantics: slots by stable order of
    # key=batch*C+(label-1); negative cats wrap, slot>=LOOP / far-oob dropped
    cat = label - 1
    key = batch * C + cat
    slots = _occ_slots(key)
    valid = (slots < LOOP) & (cat >= -C) & (cat < C)
    wvals = np.where(valid, lin_w[np.clip(slots, 0, LOOP - 1)], 0.0).astype(np.float32)
    cidx = np.mod(cat, C).astype(np.int64)

    # host scatter-sum (0.04% of total FLOPs): S[b,c,:] = sum of
    # lin_w[slot]*full over the <=LOOP boxes of bucket (b,c); slots are
    # unique per bucket so per-slot fancy-index adds have no collisions
    S = np.zeros((B, C, FEAT), np.float32)
    bok = valid & (batch >= -B) & (batch < B)
    bmod = np.mod(batch, B)
    for s in range(LOOP):
        sel = bok & (slots == s)
        if np.any(sel):
            S[bmod[sel], cidx[sel]] += wvals[sel, None] * full[sel]

    newadj = X[None, :, :] + adj                               # [B, C, C]
    has_gcb = bool(np.any(gc_b))
    KN = 6 if has_gcb else 5

    # gc_w n-major quads: gcwn[n,q,p,512*q'+c] = gc_w[(4q+q')*128+p, 512n+c]
    gcwn = np.ascontiguousarray(
        gc_w[0:2048].reshape(4, 4, 128, NCH, 512).transpose(3, 0, 2, 1, 4)
        .reshape(NCH, 4, 128, OUT)).astype(np_bf16)
    # shared phase-3 extras rows: lin_b*colsum(W_full), W_bbox[, gc_b]
    extr = np.empty((KN, OUT), np.float32)
    extr[0] = lin_b * gc_w.sum(axis=0)
    extr[1:5] = gc_w[2048:FEAT]
    if has_gcb:
        extr[5] = gc_b

    in_maps = []
    for core in range(NCORES):
        imgs = slice(core * BPC, (core + 1) * BPC)
        Xc = S[imgs].reshape(ROWS, FEAT)
        XT = np.ascontiguousarray(Xc[:, 0:2048].T)             # [2048, 800]
        xtp = np.ascontiguousarray(XT.reshape(NKC, 128, ROWS)).astype(np_bf16)
        # phase-3 stationary blocks, one [121, 100] column block per
        # (image, tile) piece: adjacency weights for the image's categories
        # at their in-tile row positions; extras weights (rowsum(A),
        # (A@x_bbox)^T[, ones]) at rows sz_t.. on the image's first piece
        Ac = newadj[imgs]                                      # [8, 100, 100]
        Sbb = S[imgs, :, 2048:FEAT]                            # [8, 100, 4]
        adjB = np.zeros((127, NPIECE * C), np.float32)
        for b in range(BPC):
            A_b = Ac[b]
            for idx, (t, j, r0, r1) in enumerate(PIECES[b]):
                cols = slice(j * C, (j + 1) * C)
                c0 = TOFF[t] + r0 - b * C
                # stat[r, i] = A_b[i, cat(r)]
                adjB[r0:r1, cols] = A_b[:, c0:c0 + (r1 - r0)].T
                if idx == 0:
                    sz = TSZ[t]
                    adjB[sz, cols] = A_b.sum(axis=1)
                    adjB[sz + 1:sz + 5, cols] = (A_b @ Sbb[b]).T
                    if has_gcb:
                        adjB[sz + 5, cols] = 1.0
        im = dict(
            gcwn=gcwn,
            xtp=xtp,
            adjB=adjB.astype(np_bf16),
            extr=extr.astype(np_bf16),
            gT=np.ascontiguousarray(gf[imgs].T).astype(np_bf16),
        )
        in_maps.append(im)

    nc = _get_program(has_gcb)
    res = None
    for attempt in range(4):
        try:
            res = bass_utils.run_bass_kernel_spmd(
                nc, in_maps, core_ids=list(range(NCORES)))
            break
        except Exception:
            if attempt == 3:
                raise
            time.sleep(3 * (attempt + 1))  # transient NRT exec-unit errors
    last_results = res
    return np.concatenate([res.results[i]["out"] for i in range(NCORES)], axis=0)


# revision 63
# speedup vs baseline: 1.1108x; 1.1108x over previous
"""GCN-Attention kernel for Trainium2, data-parallel over 8 NeuronCores.

Reference computation (per image b of 64, category c of 100):
  full = concat(image_features, bbox)                    [N, 2052]
  x[b,c,:] = sum_{boxes n in bucket(b,c), slot<3} lin_w[slot]*full[n] + lin_b
  support  = x @ gc_w                                    [B, 100, 2048]
  gcn      = leaky_relu((X + adj) @ support + gc_b)
  out[b]   = global_features[b] @ gcn[b]                 [B, 2048]

Host prep (pure input reorganization, <0.3% of total FLOPs): the occurrence-
slot scatter is resolved into the weighted sum x on the host.

Algebraic restructure: the bbox columns (4) and the lin_b bias do NOT get
their own phase-2 contraction chunk.  Because
  A_b @ (x_bbox_b @ W_bbox) = (A_b @ x_bbox_b) @ W_bbox          (rank 4)
  A_b @ (lin_b * ones ⊗ colsum(W)) = lin_b * rowsum(A_b) ⊗ colsum(W)
both fold into phase 3 as 5 extra contraction rows, costing zero extra
matmuls.  Phase 2 contracts exactly K=2048 = 16 full 128-chunks.

Row packing: the 800 (image,category) rows pack into 7 partition tiles
[100,100,120x5] (matmul cost is set by the moving free dim only, so fewer
output row-tiles = fewer matmuls: 7x16x4=448 vs per-image 512; boundaries
at 100/200 are image-aligned so only 4 images straddle tiles -> 12
phase-3 pieces).  Each tile keeps 5-6 spare partitions holding the shared
extras rows [lin_b*colsum(W); W_bbox], so phase 3 streams a tile's rows
0..sz+KN as moving operand (base partition 0 — HW requires partition-0-
aligned operand access) against a per-(image,tile) stationary block
carrying the adjacency weights for that image's categories in that tile
(zeros elsewhere, extras weights on the image's first tile only).

Phases per 512-col chunk n (4 chunks):
  phase 2: tile groups {0..3} then {4..6}: per tile, 16 K=128 chunks
           x^T (stationary) x gc_w (moving), interleaved PSUM chains.
  phase 3: per image, 1-2 accumulating matmuls (K<=126) + scalar Lrelu.
  phase 4: attention row matmul [K=100 -> 1, 512], DVE copy, 2KB DMA out.
  phase-3/4 items pop between phase-2 K-chunks (from k=3), slot-gated so
  a p4 never pops before its p3's Lrelu drains (an unready matmul stalls
  the in-order PE queue); the first batch is deferred to the third unit
  so contention-delayed constant DMAs can never stall early pops; the
  final unit runs tile-major with inline item emission so the drain
  pipelines instead of trailing.  The scalar activation table is warmed
  with a dummy Lrelu at startup (a mid-run ACT_TABLE_LOAD blocks casts).

DMA: 3 rings (sync/scalar/gpsimd), consumption-ordered; first-needed
pieces split small (contiguous dram sources only — strided sources crawl)
so the first matmul issues ~4us after the engine preamble; later gc_w
quads ship whole, round-robin.  Cross-core HBM contention varies run to
run; the schedule is arranged so no popped item can block the PE queue on
a delayed transfer.
"""
import os
import time

import ml_dtypes
import numpy as np

import concourse.bacc as bacc
import concourse.mybir as mybir
import concourse.tile as tile
from concourse import bass_utils

B = 64
C = 100
LOOP = 3
FEAT = 2052
OUT = 2048
NCORES = 8
BPC = B // NCORES  # images per core
ROWS = BPC * C     # (image,category) rows per core
NKC = 16           # K chunks of 128 (image-feature contraction only)
NCH = 4            # 512-col output chunks
NT = 7             # packed row tiles; boundaries at 100/200 are image-
                   # aligned, so only 4 images straddle tiles (12 pieces)
TSZ = [100, 100, 120, 120, 120, 120, 120]
TOFF = [0, 100, 200, 320, 440, 560, 680]

f32 = mybir.dt.float32
bf16 = mybir.dt.bfloat16
np_bf16 = ml_dtypes.bfloat16

_programs: dict = {}
last_results = None  # BassKernelResults of the most recent run (for harnesses)


def _pieces():
    """Per image: list of (tile, global piece index, row range in tile).

    Piece j's stationary block lives at columns [100j, 100j+100) of adjB.
    """
    out = []
    j = 0
    for b in range(BPC):
        g0, g1 = b * C, (b + 1) * C
        ps = []
        for t in range(NT):
            lo, hi = max(g0, TOFF[t]), min(g1, TOFF[t] + TSZ[t])
            if lo < hi:
                ps.append((t, j, lo - TOFF[t], hi - TOFF[t]))
                j += 1
        out.append(ps)
    return out


PIECES = _pieces()
NPIECE = sum(len(p) for p in PIECES)  # 14


def _occ_slots(key):
    """Occurrence index among equal-valued keys, stable order (matches jax ref)."""
    n = key.shape[0]
    order = np.argsort(key, kind="stable")
    sk = key[order]
    idx = np.arange(n)
    is_new = np.concatenate([[True], sk[1:] != sk[:-1]]) if n else np.zeros(0, bool)
    run_start = np.maximum.accumulate(np.where(is_new, idx, 0))
    pos = idx - run_start
    slots = np.zeros(n, np.int64)
    slots[order] = pos
    return slots


def _build(has_gcb: bool):
    nc = bacc.Bacc("TRN2", target_bir_lowering=False, debug=False,
                   num_devices=NCORES)

    KN = 6 if has_gcb else 5   # extras rows per tile

    gcwn_d = nc.dram_tensor("gcwn", [NCH, 4, 128, OUT], bf16, kind="ExternalInput").ap()
    xtp_d = nc.dram_tensor("xtp", [NKC, 128, ROWS], bf16, kind="ExternalInput").ap()
    adjB_d = nc.dram_tensor("adjB", [127, NPIECE * C], bf16, kind="ExternalInput").ap()
    extr_d = nc.dram_tensor("extr", [KN, OUT], bf16, kind="ExternalInput").ap()
    gT_d = nc.dram_tensor("gT", [C, BPC], bf16, kind="ExternalInput").ap()
    out_d = nc.dram_tensor("out", [BPC, OUT], f32, kind="ExternalOutput").ap()

    T0 = [0, 1, 2, 3]
    T1 = [4, 5, 6]
    # images whose pieces are all within T0 tiles / needing T1 casts
    B0 = [b for b in range(BPC) if all(t <= 3 for t, _, _, _ in PIECES[b])]
    B1 = [b for b in range(BPC) if b not in B0]

    with tile.TileContext(nc) as tc:
        with tc.tile_pool(name="const", bufs=1) as cpool, \
             tc.tile_pool(name="sb", bufs=1) as pool, \
             tc.tile_pool(name="ps", bufs=1, space="PSUM") as psp:

            R = [nc.sync, nc.scalar, nc.gpsimd]

            # ---- SBUF tiles ----
            xtp_sb = [cpool.tile([128, ROWS], bf16, tag=f"xtp{k}",
                                 name=f"xtp_sb{k}") for k in range(NKC)]
            gcwn_sb = [[cpool.tile([128, OUT], bf16, tag=f"gcwn{n}{q}",
                                   name=f"gcwn_sb{n}{q}") for q in range(4)]
                       for n in range(NCH)]
            adjB_sb = cpool.tile([127, NPIECE * C], bf16, tag="adjB",
                                 name="adjB_sb")
            gT_sb = cpool.tile([C, BPC], bf16, tag="gT", name="gT_sb")
            # packed support tiles: rows 0..sz-1 = phase-2 cast; rows
            # sz..sz+KN-1 = shared extras, DMA'd once
            ssbs = [pool.tile([TSZ[t] + KN, OUT], bf16, tag="ssb", bufs=NT,
                              name=f"ssb_{t}") for t in range(NT)]
            gsbs = [pool.tile([C, OUT], bf16, tag="gsb", bufs=BPC,
                              name=f"gsb_{b}") for b in range(BPC)]

            # warm the scalar activation table with Lrelu during the
            # DMA-bound startup; otherwise the first real Lrelu (~45us)
            # triggers a 1.3us ACT_TABLE_LOAD that blocks casts and the
            # popped phase-3/4 chain behind it
            warm = pool.tile([1, 32], f32, tag="warm", name="warm")
            nc.vector.memset(warm[0:1, 0:32], 0.0)
            nc.scalar.activation(warm[0:1, 0:32], warm[0:1, 0:32],
                                 mybir.ActivationFunctionType.Lrelu,
                                 alpha=0.01)

            # warm the PE while the first input DMAs land: ~9 dummy
            # matmuls (3.8us of cold-rate activity) flip the HAM clock
            # gate to 8/8 so the first real matmuls run at 2.4GHz instead
            # of paying the 1.2GHz cold window
            wstat = pool.tile([128, 128], bf16, tag="wstat", name="wstat")
            nc.vector.memset(wstat[0:128, 0:128], 0.0)
            wmov = pool.tile([128, 512], bf16, tag="wmov", name="wmov")
            nc.vector.memset(wmov[0:128, 0:512], 0.0)
            wps = psp.tile([128, 512], f32, tag="gp", bufs=2, name="wps")
            for i in range(9):
                nc.tensor.matmul(wps[0:128, 0:512], wstat[0:128, 0:128],
                                 wmov[0:128, 0:512],
                                 start=(i == 0), stop=(i == 8))

            # ---- DMA delivery, consumption-ordered ----
            nc.sync.dma_start(xtp_sb[0][0:64, :], xtp_d[0, 0:64])
            nc.scalar.dma_start(xtp_sb[0][64:128, :], xtp_d[0, 64:128])
            nc.gpsimd.dma_start(gcwn_sb[0][0][:, 0:512], gcwn_d[0, 0, :, 0:512])
            nc.sync.dma_start(gcwn_sb[0][0][:, 512:1024],
                              gcwn_d[0, 0, :, 512:1024])
            nc.scalar.dma_start(gcwn_sb[0][0][:, 1024:1536],
                                gcwn_d[0, 0, :, 1024:1536])
            nc.gpsimd.dma_start(gcwn_sb[0][0][:, 1536:2048],
                                gcwn_d[0, 0, :, 1536:2048])
            nc.sync.dma_start(xtp_sb[1][:], xtp_d[1])
            nc.scalar.dma_start(xtp_sb[2][:], xtp_d[2])
            nc.gpsimd.dma_start(xtp_sb[3][:], xtp_d[3])
            nc.sync.dma_start(gcwn_sb[0][1][:, 0:1024], gcwn_d[0, 1, :, 0:1024])
            nc.scalar.dma_start(gcwn_sb[0][1][:, 1024:2048],
                                gcwn_d[0, 1, :, 1024:2048])
            for k in range(4, 7):
                R[k % 3].dma_start(xtp_sb[k][:], xtp_d[k])
            nc.gpsimd.dma_start(gcwn_sb[0][2][:, 0:1024], gcwn_d[0, 2, :, 0:1024])
            nc.sync.dma_start(gcwn_sb[0][2][:, 1024:2048],
                              gcwn_d[0, 2, :, 1024:2048])
            for k in range(7, 10):
                R[k % 3].dma_start(xtp_sb[k][:], xtp_d[k])
            nc.scalar.dma_start(gcwn_sb[0][3][:, 0:1024], gcwn_d[0, 3, :, 0:1024])
            nc.gpsimd.dma_start(gcwn_sb[0][3][:, 1024:2048],
                                gcwn_d[0, 3, :, 1024:2048])
            for k in range(10, NKC):
                R[k % 3].dma_start(xtp_sb[k][:], xtp_d[k])
            # phase-3/4 constants (first needed when the first phase-3 item
            # pops, ~k=3 of the third unit).  All extras ride the gpsimd
            # ring: the software DGE issues without blocking, so they can
            # never queue behind a sem-reuse chain on a busy engine
            nc.sync.dma_start(adjB_sb[:], adjB_d[:])
            nc.gpsimd.dma_start(gT_sb[:], gT_d[:])
            for t in range(NT):
                nc.gpsimd.dma_start(ssbs[t][TSZ[t]:TSZ[t] + KN, :], extr_d[:])
            # remaining gc_w quads: n=1 in halves, n=2/3 whole, round-robin
            qi = 0
            for q in range(4):
                R[qi % 3].dma_start(gcwn_sb[1][q][:, 0:1024],
                                    gcwn_d[1, q, :, 0:1024])
                R[(qi + 1) % 3].dma_start(gcwn_sb[1][q][:, 1024:2048],
                                          gcwn_d[1, q, :, 1024:2048])
                qi += 2
            for n in range(2, NCH):
                for q in range(4):
                    R[qi % 3].dma_start(gcwn_sb[n][q][:], gcwn_d[n, q])
                    qi += 1

            def stat_slice(k, t):
                return xtp_sb[k][0:128, TOFF[t]:TOFF[t] + TSZ[t]]

            def mov_slice(k, n):
                gt = gcwn_sb[n][k // 4]
                return gt[0:128, (k % 4) * 512:(k % 4) * 512 + 512]

            def cast(i, dst, src):
                # PSUM -> SBUF bf16 drain, spread across two engines
                eng = (nc.vector, nc.scalar, nc.vector, nc.scalar)[i % 4]
                if eng is nc.scalar:
                    eng.activation(dst, src, mybir.ActivationFunctionType.Copy)
                else:
                    eng.tensor_copy(dst, src)

            def walk(g, n, pop_item):
                # tiles of g x 16 K chunks at a fixed 512-col block:
                # interleaved PSUM chains; pending phase-3/4 items are
                # sandwiched between K chunks (from k=3, so the previous
                # unit's casts have drained) so the PE never idles
                chains = [psp.tile([128, 512], f32, tag="ch", bufs=6,
                                   name=f"ch_{t}_{n}") for t in g]
                for k in range(NKC):
                    for i, t in enumerate(g):
                        nc.tensor.matmul(
                            chains[i][0:TSZ[t], 0:512],
                            stat_slice(k, t),
                            mov_slice(k, n),
                            start=(k == 0), stop=(k == NKC - 1),
                        )
                    if k >= 3:
                        pop_item()
                for i, t in enumerate(g):
                    cast(i, ssbs[t][0:TSZ[t], n * 512:(n + 1) * 512],
                         chains[i][0:TSZ[t], 0:512])

            def walk_tmajor(g, n, pop_item, enq):
                # steady-state units (all data resident): tile-major — 16
                # back-to-back accumulating matmuls on one bank, casting
                # each tile as it completes and enqueueing the phase-3/4 of
                # images it finishes; casts spread one-per-tile instead of
                # bunching at unit boundaries (where chain-start matmuls
                # would wait on them for PSUM bank reuse)
                chains = [psp.tile([128, 512], f32, tag="ch", bufs=6,
                                   name=f"chl_{t}_{n}") for t in g]
                for i, t in enumerate(g):
                    for k in range(NKC):
                        nc.tensor.matmul(
                            chains[i][0:TSZ[t], 0:512],
                            stat_slice(k, t),
                            mov_slice(k, n),
                            start=(k == 0), stop=(k == NKC - 1),
                        )
                        if k % 3 == 2:
                            pop_item()
                    cast(i, ssbs[t][0:TSZ[t], n * 512:(n + 1) * 512],
                         chains[i][0:TSZ[t], 0:512])
                    enq(t, n)

            def p3_item(b, n):
                # G[b][:,n] = Lrelu(sum over pieces of block contraction)
                gp = psp.tile([128, 512], f32, tag="gp", bufs=2,
                              name=f"gp_{b}_{n}")
                ps = PIECES[b]
                for idx, (t, j, _, _) in enumerate(ps):
                    kk = TSZ[t] + KN
                    nc.tensor.matmul(
                        gp[0:C, 0:512],
                        adjB_sb[0:kk, j * C:(j + 1) * C],
                        ssbs[t][0:kk, n * 512:(n + 1) * 512],
                        start=(idx == 0), stop=(idx == len(ps) - 1),
                    )
                nc.scalar.activation(
                    gsbs[b][0:C, n * 512:(n + 1) * 512],
                    gp[0:C, 0:512],
                    mybir.ActivationFunctionType.Lrelu, alpha=0.01,
                )

            def p4_item(b, n):
                op = psp.tile([128, 512], f32, tag="gp", bufs=2,
                              name=f"op_{b}_{n}")
                nc.tensor.matmul(op[0:1, 0:512],
                                 gT_sb[0:C, b:b + 1],
                                 gsbs[b][0:C, n * 512:(n + 1) * 512],
                                 start=True, stop=True)
                ost = pool.tile([1, 512], f32, tag="ostage", bufs=4,
                                name=f"ost_{b}_{n}")
                nc.vector.tensor_copy(ost[0:1, 0:512], op[0:1, 0:512])
                R[b % 3].dma_start(out_d[b:b + 1, n * 512:(n + 1) * 512],
                                   ost[0:1, 0:512])

            queue = []
            slot = [0]

            def pop_item():
                # items gate on a minimum slot so a p4 never pops before its
                # p3's Lrelu (~1.3us) has drained — an unready p4 matmul
                # stalls the whole in-order PE queue
                slot[0] += 1
                if queue and queue[0][0] <= slot[0]:
                    _, kind, b, n = queue.pop(0)
                    if kind == 3:
                        p3_item(b, n)
                        queue.append((slot[0] + 2, 4, b, n))
                    else:
                        p4_item(b, n)

            # image is ready once its last tile is cast
            last_tile = {b: max(t for t, _, _, _ in PIECES[b])
                         for b in range(BPC)}

            def enq(t, n):
                for b in range(BPC):
                    if last_tile[b] == t:
                        queue.append((slot[0] + 1, 3, b, n))

            units = [(g, n) for n in range(NCH) for g in (T0, T1)]
            for u, (g, n) in enumerate(units):
                # the first phase-3 batch waits until u=2 (~40us) so a
                # contention-delayed adjB/extras DMA can never stall the
                # in-order PE queue through an early popped item
                if u == 2:
                    queue.extend((0, 3, b, 0) for b in B0)
                    queue.extend((0, 3, b, 0) for b in B1)
                elif u >= 3:
                    gp_, np_ = units[u - 1]
                    if gp_ is T0:
                        queue.extend((0, 3, b, np_) for b in B0)
                    else:
                        queue.extend((0, 3, b, np_) for b in B1)
                if u == len(units) - 1:
                    walk_tmajor(g, n, pop_item, enq)
                else:
                    walk(g, n, pop_item)
            while queue:
                pop_item()

    nc.compile()
    return nc


def _get_program(has_gcb: bool = False):
    key = ("pack115", has_gcb)
    if key not in _programs:
        _programs[key] = _build(has_gcb)
    return _programs[key]


def kernel(**inputs) -> np.ndarray:
    global last_results

    imf = np.asarray(inputs["image_features"], np.float32)
    bbox = np.asarray(inputs["bbox_list"], np.float32)
    gf = np.asarray(inputs["global_features"], np.float32)
    adj = np.asarray(inputs["adj"], np.float32)
    X = np.asarray(inputs["X"], np.float32)
    lin_w = np.asarray(inputs["lin_w"], np.float32)
    lin_b = np.float32(np.asarray(inputs["lin_b"]))
    gc_w = np.ascontiguousarray(np.asarray(inputs["gc_w"], np.float32))
    gc_b = np.asarray(inputs["gc_b"], np.float32)
    label = np.asarray(inputs["label_list"]).astype(np.int64)
    batch = np.asarray(inputs["batch"]).astype(np.int64)

    full = np.concatenate([imf, bbox], axis=1)

    # scatter bookkeeping, matching jax semantics: slots by stable order of
    # key=batch*C+(label-1); negative cats wrap, slot>=LOOP / far-oob dropped
    cat = label - 1
    key = batch * C + cat
    slots = _occ_slots(key)
    valid = (slots < LOOP) & (cat >= -C) & (cat < C)
    wvals = np.where(valid, lin_w[np.clip(slots, 0, LOOP - 1)], 0.0).astype(np.float32)
    cidx = np.mod(cat, C).astype(np.int64)

    # host scatter-sum (0.04% of total FLOPs): S[b,c,:] = sum of
    # lin_w[slot]*full over the <=LOOP boxes of bucket (b,c); slots are
    # unique per bucket so per-slot fancy-index adds have no collisions
    S = np.zeros((B, C, FEAT), np.float32)
    bok = valid & (batch >= -B) & (batch < B)
    bmod = np.mod(batch, B)
    for s in range(LOOP):
        sel = bok & (slots == s)
        if np.any(sel):
            S[bmod[sel], cidx[sel]] += wvals[sel, None] * full[sel]

    newadj = X[None, :, :] + adj                               # [B, C, C]
    has_gcb = bool(np.any(gc_b))
    KN = 6 if has_gcb else 5

    # gc_w n-major quads: gcwn[n,q,p,512*q'+c] = gc_w[(4q+q')*128+p, 512n+c]
    gcwn = np.ascontiguousarray(
        gc_w[0:2048].reshape(4, 4, 128, NCH, 512).transpose(3, 0, 2, 1, 4)
        .reshape(NCH, 4, 128, OUT)).astype(np_bf16)
    # shared phase-3 extras rows: lin_b*colsum(W_full), W_bbox[, gc_b]
    extr = np.empty((KN, OUT), np.float32)
    extr[0] = lin_b * gc_w.sum(axis=0)
    extr[1:5] = gc_w[2048:FEAT]
    if has_gcb:
        extr[5] = gc_b

    in_maps = []
    for core in range(NCORES):
        imgs = slice(core * BPC, (core + 1) * BPC)
        Xc = S[imgs].reshape(ROWS, FEAT)
        XT = np.ascontiguousarray(Xc[:, 0:2048].T)             # [2048, 800]
        xtp = np.ascontiguousarray(XT.reshape(NKC, 128, ROWS)).astype(np_bf16)
        # phase-3 stationary blocks, one [121, 100] column block per
        # (image, tile) piece: adjacency weights for the image's categories
        # at their in-tile row positions; extras weights (rowsum(A),
        # (A@x_bbox)^T[, ones]) at rows sz_t.. on the image's first piece
        Ac = newadj[imgs]                                      # [8, 100, 100]
        Sbb = S[imgs, :, 2048:FEAT]                            # [8, 100, 4]
        adjB = np.zeros((127, NPIECE * C), np.float32)
        for b in range(BPC):
            A_b = Ac[b]
            for idx, (t, j, r0, r1) in enumerate(PIECES[b]):
                cols = slice(j * C, (j + 1) * C)
                c0 = TOFF[t] + r0 - b * C
                # stat[r, i] = A_b[i, cat(r)]
                adjB[r0:r1, cols] = A_b[:, c0:c0 + (r1 - r0)].T
                if idx == 0:
                    sz = TSZ[t]
                    adjB[sz, cols] = A_b.sum(axis=1)
                    adjB[sz + 1:sz + 5, cols] = (A_b @ Sbb[b]).T
                    if has_gcb:
                        adjB[sz + 5, cols] = 1.0
        im = dict(
            gcwn=gcwn,
            xtp=xtp,
            adjB=adjB.astype(np_bf16),
            extr=extr.astype(np_bf16),
            gT=np.ascontiguousarray(gf[imgs].T).astype(np_bf16),
        )
        in_maps.append(im)

    nc = _get_program(has_gcb)
    res = None
    for attempt in range(4):
        try:
            res = bass_utils.run_bass_kernel_spmd(
                nc, in_maps, core_ids=list(range(NCORES)))
            break
        except Exception:
            if attempt == 3:
                raise
            time.sleep(3 * (attempt + 1))  # transient NRT exec-unit errors
    last_results = res
    return np.concatenate([res.results[i]["out"] for i in range(NCORES)], axis=0)
